# revision 2
# baseline (speedup 1.0000x reference)
"""Trainium2 Bass kernel for a 2-layer GCN (gnn_message_passing).

Reference computation (all f32 inputs):
    h      = relu(adj @ (x @ W1) + b1)        adj: [N, N], x: [N, F]
    logits = adj @ (h @ W2) + b2
    out    = log_softmax(logits, axis=1)       out: [N, C]

Distribution: 1-D row partition over 8 NeuronCores. Core i owns rows
R0 = i*N/8 .. R0+N/8. Because adj is symmetric (by construction), the
column slice adj[:, rows_i] in natural row-major layout is exactly the
transposed operand adj_i^T the TensorEngine needs as its moving operand,
so no on-chip transpose of adj is ever required.

Per-core plan (single NEFF launch, two AllGathers):
  - adj[:, rows_i] streamed via SWDGE cast-DMAs (f32 -> bf16 in flight,
    ~358 GB/s): host pre-permutes rows so each partition's slice is one
    16KB-contiguous descriptor. adj kept RESIDENT in SBUF (16MB bf16)
    so layer 2 re-uses it with zero extra HBM traffic.
  - S_i = x_i @ W1 from a host-transposed x_i^T (no PE transposes),
    AllGather S (bf16, first + only collective before layer 1) issued
    ~15us in; the collective runs under the adj stream.
  - layer 1: hT[f, m] accumulated in PSUM over all 64 k-chunks as the
    adj chunks land (DMA-gated, PE never idles > the chunk gap).
  - z_i = h_i @ W2 -> AllGather z (bf16, tiny), PE warm-keeper spans the
    collective gap so layer 2 starts at full clock.
  - layer 2: logitsT[c, m] from resident adj + gathered z, +b2,
    PE-transpose to [m, c], log_softmax on-chip, single output DMA.

kernel(**inputs) takes FULL inputs and returns the FULL [N, C] output.
"""

import numpy as np

import concourse.bass as bass
import concourse.mybir as mybir
import concourse.tile as tile
from concourse import bacc
from concourse.bass_utils import run_bass_kernel_spmd
from concourse.masks import make_identity

NCORES = 8
N_FULL = 8192
NFEAT = 512
NHID = 128
NCLASS = 40
F32 = mybir.dt.float32
BF16 = mybir.dt.bfloat16

KK = 4            # adj k-chunks per cast-DMA (16KB contiguous per partition)
WARM_N = 150      # PE warm-keeper matmuls (N=128) spanning the z-AllGather
L2_COLTILE = False


def build(n_total: int = N_FULL):
    """Build the SPMD Bass graph for one core (same program on all 8)."""
    M = n_total // NCORES          # rows owned by this core
    K = n_total // 128             # 128-row contraction chunks
    MC = M // 128                  # 128-row output chunks on this core
    MW = min(512, M)               # moving free-dim width for the big matmuls
    MH = M // MW                   # number of row groups of width MW
    DF = NFEAT // 128              # feature chunks (4)

    nc = bacc.Bacc(
        "TRN2", target_bir_lowering=False, debug=False,
        enable_asserts=True, num_devices=NCORES,
    )

    xt = nc.dram_tensor("xt", [NFEAT, M], F32, kind="ExternalInput")
    adjc = nc.dram_tensor("adjc", [n_total, M], F32, kind="ExternalInput")
    w1 = nc.dram_tensor("w1", [NFEAT, NHID], F32, kind="ExternalInput")
    b1 = nc.dram_tensor("b1", [NHID, 1], F32, kind="ExternalInput")
    w2 = nc.dram_tensor("w2", [NHID, NCLASS], F32, kind="ExternalInput")
    b2 = nc.dram_tensor("b2", [NCLASS, 1], F32, kind="ExternalInput")
    out_ext = nc.dram_tensor("out", [M, NCLASS], F32, kind="ExternalOutput")

    rg = [list(range(NCORES))]

    with tile.TileContext(nc) as tc:
        with (
            tc.tile_pool(name="resident", bufs=1) as res,
            tc.tile_pool(name="dram", bufs=1, space="DRAM") as dram,
        ):
            adjres = res.tile([128, K * M], BF16)          # adj_i^T, bf16, resident
            sres = res.tile([128, K, NHID], BF16)          # gathered S, k-chunk layout
            zres = res.tile([128, K, NCLASS], BF16)        # gathered z, k-chunk layout
            hT = res.tile([128, M], BF16)                  # layer-1 out, [f, m]
            xts = res.tile([128, DF, M], F32)              # x_i^T staged f32
            xtb = res.tile([128, DF, M], BF16)             # x_i^T bf16
            sloc = res.tile([128, MC, NHID], BF16)         # local S rows
            zloc = res.tile([128, MC, NCLASS], BF16)
            w1st = res.tile([128, DF, NHID], F32)
            w1bf = res.tile([128, DF, NHID], BF16)
            w2st = res.tile([128, NCLASS], F32)
            w2bf = res.tile([128, NCLASS], BF16)
            b1sb = res.tile([128, 1], F32)
            b2sb = res.tile([NCLASS, 1], F32)
            ident = res.tile([128, 128], F32)
            lTsb = res.tile([NCLASS, M], F32)              # logits^T (+b2)
            osb = res.tile([128, MC, NCLASS], F32)         # final log-softmax out

            # collective bounce buffers (internal DRAM)
            s_in = dram.tile([M, NHID], BF16)
            s_out = dram.tile([n_total, NHID], BF16, addr_space="Shared")
            z_in = dram.tile([M, NCLASS], BF16)
            z_out = dram.tile([n_total, NCLASS], BF16, addr_space="Shared")

            # ---- adj stream: SWDGE cast-DMAs (f32->bf16 in flight), issued
            # first so the 16 SDMA engines stream adj from t~2us. Host
            # permutes rows so partition p's slice of each superchunk is
            # KK*4KB contiguous in HBM. ----
            for kk in range(K // KK):
                nc.gpsimd.dma_start(
                    out=adjres[:, kk * KK * M:(kk + 1) * KK * M].rearrange(
                        "p (a m) -> p a m", a=KK
                    ),
                    in_=adjc[kk * KK * 128:(kk + 1) * KK * 128, :].rearrange(
                        "(p a) m -> p a m", p=128
                    ),
                )

            # ---- constants + x^T on the sync HWDGE queue (gpsimd is busy
            # generating adj descriptors; sync is otherwise idle early) ----
            nc.sync.dma_start(
                out=w1st[:, :, :],
                in_=w1.ap().rearrange("(a p) f -> p a f", p=128),
            )
            nc.sync.dma_start(
                out=xts[:, :, :],
                in_=xt.ap().rearrange("(a p) m -> p a m", p=128),
            )
            nc.sync.dma_start(out=b1sb[:, :], in_=b1.ap())
            nc.sync.dma_start(out=b2sb[:, :], in_=b2.ap())
            nc.sync.dma_start(out=w2st[:, :], in_=w2.ap())
            nc.vector.tensor_copy(w1bf[:, :, :], w1st[:, :, :])
            nc.vector.tensor_copy(xtb[:, :, :], xts[:, :, :])
            nc.vector.tensor_copy(w2bf[:, :], w2st[:, :])
            make_identity(nc, ident[:, :])

            # ---- S phase: S_i = x_i @ W1 (stationary x^T chunks) ----
            with tc.tile_pool(name="spsum", bufs=2, space="PSUM") as spsum:
                for nci in range(MC):
                    ps = spsum.tile([128, NHID], F32, tag="ps")
                    for d in range(DF):
                        nc.tensor.matmul(
                            ps[:, :],
                            xtb[:, d, nci * 128:(nci + 1) * 128],
                            w1bf[:, d, :],
                            start=(d == 0), stop=(d == DF - 1),
                        )
                    nc.vector.tensor_copy(sloc[:, nci, :], ps[:, :])
            nc.sync.dma_start(
                out=s_in.rearrange("(a p) f -> p a f", p=128),
                in_=sloc[:, :, :],
            )
            nc.gpsimd.collective_compute(
                "AllGather", mybir.AluOpType.bypass, replica_groups=rg,
                ins=[s_in[:, :]], outs=[s_out[:, :]],
            )
            nc.sync.dma_start(
                out=sres[:, :, :],
                in_=s_out.rearrange("(k p) f -> p k f", p=128),
            )

            # ---- layer 1: hT += S_k^T @ adjT_k (DMA-gated on adj chunks) ----
            with tc.tile_pool(name="hpsum", bufs=1, space="PSUM") as hp:
                ph = [hp.tile([128, MW], F32, name=f"ph{m}") for m in range(MH)]
                for k in range(K):
                    for mh in range(MH):
                        nc.tensor.matmul(
                            ph[mh][:, :],
                            sres[:, k, :],
                            adjres[:, k * M + mh * MW:k * M + (mh + 1) * MW],
                            start=(k == 0), stop=(k == K - 1),
                        )
                for mh in range(MH):
                    nc.scalar.activation(
                        hT[:, mh * MW:(mh + 1) * MW], ph[mh][:, :],
                        mybir.ActivationFunctionType.Relu,
                        bias=b1sb[:, 0:1], scale=1.0,
                    )

            # ---- z_i = h_i @ W2 ----
            with tc.tile_pool(name="zpsum", bufs=2, space="PSUM") as zp:
                for mc in range(MC):
                    pz = zp.tile([128, NCLASS], F32, tag="pz")
                    nc.tensor.matmul(
                        pz[:, :],
                        hT[:, mc * 128:(mc + 1) * 128],
                        w2bf[:, :],
                        start=True, stop=True,
                    )
                    nc.vector.tensor_copy(zloc[:, mc, :], pz[:, :])
            nc.sync.dma_start(
                out=z_in.rearrange("(a p) c -> p a c", p=128),
                in_=zloc[:, :, :],
            )

            # ---- PE warm-keeper: discardable matmuls spanning the z-AllGather
            # gap so HAM keeps the PE at full clock for layer 2. ----
            with tc.tile_pool(name="wpsum", bufs=1, space="PSUM") as wp:
                wps = wp.tile([128, 128], F32)
                for i in range(WARM_N):
                    nc.tensor.matmul(
                        wps[:, :],
                        w1bf[:, 0, :],
                        hT[:, 0:128],
                        start=True, stop=True,
                    )

            nc.gpsimd.collective_compute(
                "AllGather", mybir.AluOpType.bypass, replica_groups=rg,
                ins=[z_in[:, :]], outs=[z_out[:, :]],
            )
            nc.sync.dma_start(
                out=zres[:, :, :],
                in_=z_out.rearrange("(k p) c -> p k c", p=128),
            )

            # ---- layer 2: logitsT += z_k^T @ adjT_k ----
            with tc.tile_pool(name="lpsum", bufs=1, space="PSUM") as lp:
                pl = [lp.tile([NCLASS, MW], F32, name=f"pl{m}") for m in range(MH)]
                for k in range(K):
                    for mh in range(MH):
                        nc.tensor.matmul(
                            pl[mh][:, :],
                            zres[:, k, :],
                            adjres[:, k * M + mh * MW:k * M + (mh + 1) * MW],
                            start=(k == 0), stop=(k == K - 1),
                        )
                for mh in range(MH):
                    nc.scalar.activation(
                        lTsb[:, mh * MW:(mh + 1) * MW], pl[mh][:, :],
                        mybir.ActivationFunctionType.Identity,
                        bias=b2sb[:, 0:1], scale=1.0,
                    )

            # ---- log_softmax over classes, batched per activation function so
            # the scalar engine loads each ACT table once ----
            with (
                tc.tile_pool(name="smp", bufs=1, space="PSUM") as smp,
                tc.tile_pool(name="sms", bufs=1) as sms,
            ):
                ptrs = [smp.tile([128, NCLASS], F32, name=f"ptr{m}") for m in range(MC)]
                mx = sms.tile([128, MC], F32)
                ssum = sms.tile([128, MC], F32)
                lse = sms.tile([128, MC], F32)
                bias2 = sms.tile([128, MC], F32)
                esc = sms.tile([128, NCLASS], F32)
                for mc in range(MC):
                    nc.tensor.transpose(
                        ptrs[mc][:, :], lTsb[:, mc * 128:(mc + 1) * 128],
                        ident[0:NCLASS, 0:NCLASS],
                    )
                for mc in range(MC):
                    nc.vector.tensor_reduce(
                        mx[:, mc:mc + 1], ptrs[mc][:, :], axis=mybir.AxisListType.X,
                        op=mybir.AluOpType.max, negate=True,
                    )
                for mc in range(MC):
                    nc.scalar.activation(
                        esc[:, :], ptrs[mc][:, :], mybir.ActivationFunctionType.Exp,
                        bias=mx[:, mc:mc + 1], scale=1.0,
                        accum_out=ssum[:, mc:mc + 1],
                    )
                nc.scalar.activation(
                    lse[:, :], ssum[:, :], mybir.ActivationFunctionType.Ln,
                )
                nc.vector.tensor_sub(bias2[:, :], mx[:, :], lse[:, :])
                for mc in range(MC):
                    nc.vector.tensor_scalar_add(
                        osb[:, mc, :], ptrs[mc][:, :], bias2[:, mc:mc + 1],
                    )
            nc.sync.dma_start(
                out=out_ext.ap().rearrange("(a p) c -> p a c", p=128),
                in_=osb[:, :, :],
            )

    nc.compile()
    return nc


_NC_CACHE = {}


def _get_nc(n_total: int):
    if n_total not in _NC_CACHE:
        _NC_CACHE[n_total] = build(n_total)
    return _NC_CACHE[n_total]


def _permute_rows(a: np.ndarray, kk: int) -> np.ndarray:
    """Reorder rows so the device's "(p a) m" DMA layout reconstructs the
    natural "(a p) m" k-chunk layout with KK*4KB-contiguous descriptors."""
    n, m = a.shape
    nblk = n // (128 * kk)
    return np.ascontiguousarray(
        a.reshape(nblk, kk, 128, m).transpose(0, 2, 1, 3).reshape(n, m)
    )


def make_in_maps(x, adj, W1, b1, W2, b2):
    n_total = x.shape[0]
    m = n_total // NCORES
    in_maps = []
    for i in range(NCORES):
        r0 = i * m
        in_maps.append({
            "xt": np.ascontiguousarray(x[r0:r0 + m].T),
            "adjc": _permute_rows(np.ascontiguousarray(adj[:, r0:r0 + m]), KK),
            "w1": np.ascontiguousarray(W1),
            "b1": np.ascontiguousarray(b1.reshape(NHID, 1)),
            "w2": np.ascontiguousarray(W2),
            "b2": np.ascontiguousarray(b2.reshape(NCLASS, 1)),
        })
    return in_maps


def kernel(x, adj, W1, b1, W2, b2):
    x = np.asarray(x, dtype=np.float32)
    adj = np.asarray(adj, dtype=np.float32)
    W1 = np.asarray(W1, dtype=np.float32)
    b1 = np.asarray(b1, dtype=np.float32)
    W2 = np.asarray(W2, dtype=np.float32)
    b2 = np.asarray(b2, dtype=np.float32)
    nc = _get_nc(x.shape[0])
    in_maps = make_in_maps(x, adj, W1, b1, W2, b2)
    res = run_bass_kernel_spmd(nc, in_maps, list(range(NCORES)))
    return np.concatenate([res.results[i]["out"] for i in range(NCORES)], axis=0)


# revision 3
# speedup vs baseline: 1.2015x; 1.2015x over previous
"""Trainium2 Bass kernel for a 2-layer GCN (gnn_message_passing).

Reference computation (all f32 inputs):
    h      = relu(adj @ (x @ W1) + b1)        adj: [N, N], x: [N, F]
    logits = adj @ (h @ W2) + b2
    out    = log_softmax(logits, axis=1)       out: [N, C]

Distribution: 1-D row partition over 8 NeuronCores. Core i owns rows
R0 = i*N/8 .. R0+N/8. Because adj is symmetric (by construction), the
column slice adj[:, rows_i] in natural row-major layout is exactly the
transposed operand adj_i^T the TensorEngine needs as its moving operand,
so no on-chip transpose of adj is ever required.

Per-core plan (single NEFF launch, two AllGathers):
  - adj[:, rows_i] streamed via SWDGE cast-DMAs (f32 -> bf16 in flight,
    ~HBM line rate): host pre-permutes rows so each partition's slice of
    a superchunk is one KK*4KB-contiguous descriptor. adj stays RESIDENT
    in SBUF (16MB bf16) so layer 2 re-uses it with zero extra HBM traffic.
  - x_i^T loaded the same way (host-transposed, cast in flight). The
    S-AllGather trigger is interleaved into the gpsimd queue after only
    4 adj descriptor-gens so it fires ~15us in (the SWDGE ring paces
    desc-gen at drain speed; queueing it last would delay it ~100us).
  - layer 1: hT[f, m] accumulated in PSUM over all 64 k-chunks as the
    adj chunks land (DMA-gated).
  - z_i = h_i @ W2 -> AllGather z (bf16, tiny), PE warm-keeper spans the
    collective gap so layer 2 starts at full clock.
  - layer 2: logitsT from resident adj + gathered z, 2x column-tiled on
    the PE (40-wide output uses col-groups 0-1 for m-block 0 and 2-3 for
    m-block 1 concurrently -> ~2x), +b2, PE-transpose to [m, c],
    log_softmax on-chip, single output DMA.

kernel(**inputs) takes FULL inputs and returns the FULL [N, C] output.
"""

import numpy as np

import concourse.bass as bass
import concourse.mybir as mybir
import concourse.tile as tile
from concourse import bacc
from concourse.bass_utils import run_bass_kernel_spmd
from concourse.masks import make_identity

NCORES = 8
N_FULL = 8192
NFEAT = 512
NHID = 128
NCLASS = 40
F32 = mybir.dt.float32
BF16 = mybir.dt.bfloat16

KK = 4            # adj k-chunks per cast-DMA (16KB contiguous per partition)
ADJ_EARLY = 4     # adj DMAs desc-genned before the S-AllGather trigger
WARM_N = 150      # PE warm-keeper matmuls (N=128) spanning the z-AllGather
L2_COLTILE = True


def build(n_total: int = N_FULL):
    """Build the SPMD Bass graph for one core (same program on all 8)."""
    M = n_total // NCORES          # rows owned by this core
    K = n_total // 128             # 128-row contraction chunks
    MC = M // 128                  # 128-row output chunks on this core
    MW = min(512, M)               # moving free-dim width for the big matmuls
    MH = M // MW                   # number of row groups of width MW
    DF = NFEAT // 128              # feature chunks (4)

    nc = bacc.Bacc(
        "TRN2", target_bir_lowering=False, debug=False,
        enable_asserts=True, num_devices=NCORES,
    )

    xt = nc.dram_tensor("xt", [NFEAT, M], F32, kind="ExternalInput")
    adjc = nc.dram_tensor("adjc", [n_total, M], F32, kind="ExternalInput")
    w1 = nc.dram_tensor("w1", [NFEAT, NHID], F32, kind="ExternalInput")
    b1 = nc.dram_tensor("b1", [NHID, 1], F32, kind="ExternalInput")
    w2 = nc.dram_tensor("w2", [NHID, NCLASS], F32, kind="ExternalInput")
    b2 = nc.dram_tensor("b2", [NCLASS, 1], F32, kind="ExternalInput")
    out_ext = nc.dram_tensor("out", [M, NCLASS], F32, kind="ExternalOutput")

    rg = [list(range(NCORES))]

    with tile.TileContext(nc) as tc:
        with (
            tc.tile_pool(name="resident", bufs=1) as res,
            tc.tile_pool(name="dram", bufs=1, space="DRAM") as dram,
        ):
            adjres = res.tile([128, K * M], BF16)          # adj_i^T, bf16, resident
            sres = res.tile([128, K, NHID], BF16)          # gathered S, k-chunk layout
            zres = res.tile([128, K, NCLASS], BF16)        # gathered z, k-chunk layout
            hT = res.tile([128, M], BF16)                  # layer-1 out, [f, m]
            xtb = res.tile([128, DF, M], BF16)             # x_i^T bf16
            sloc = res.tile([128, MC, NHID], BF16)         # local S rows
            zloc = res.tile([128, MC, NCLASS], BF16)
            w1st = res.tile([128, DF, NHID], F32)
            w1bf = res.tile([128, DF, NHID], BF16)
            w2st = res.tile([128, NCLASS], F32)
            w2bf = res.tile([128, NCLASS], BF16)
            b1sb = res.tile([128, 1], F32)
            b2sb = res.tile([NCLASS, 1], F32)
            b2hi = res.tile([128, 1], F32)                 # b2 dup at partitions 64..103
            ident = res.tile([128, 128], F32)
            ident2 = res.tile([128, NCLASS], F32)          # shifted identity (64..103)
            lTsb = res.tile([NCLASS, M], F32)              # logits^T m-block 0 (+b2)
            lThi = res.tile([128, MW], F32)                # logits^T m-block 1 @64..103
            osb = res.tile([128, MC, NCLASS], F32)         # final log-softmax out

            # collective bounce buffers (internal DRAM)
            s_in = dram.tile([M, NHID], BF16)
            s_out = dram.tile([n_total, NHID], BF16, addr_space="Shared")
            z_in = dram.tile([M, NCLASS], BF16)
            z_out = dram.tile([n_total, NCLASS], BF16, addr_space="Shared")

            # ---- gpsimd SWDGE queue, in order: x^T cast-DMA, 4 adj
            # cast-DMAs, S-AllGather trigger, the remaining adj DMAs, and
            # (much later) the z-AllGather trigger. The SWDGE descriptor
            # ring paces desc-gen at drain speed, so the S-AG trigger must
            # not sit behind the full adj stream. ----
            nc.gpsimd.dma_start(
                out=xtb[:, :, :],
                in_=xt.ap().rearrange("(a p) m -> p a m", p=128),
            )

            def adj_dma(kk):
                nc.gpsimd.dma_start(
                    out=adjres[:, kk * KK * M:(kk + 1) * KK * M].rearrange(
                        "p (a m) -> p a m", a=KK
                    ),
                    in_=adjc[kk * KK * 128:(kk + 1) * KK * 128, :].rearrange(
                        "(p a) m -> p a m", p=128
                    ),
                )

            for kk in range(ADJ_EARLY):
                adj_dma(kk)

            # ---- constants on the sync HWDGE queue ----
            nc.sync.dma_start(
                out=w1st[:, :, :],
                in_=w1.ap().rearrange("(a p) f -> p a f", p=128),
            )
            nc.sync.dma_start(out=b1sb[:, :], in_=b1.ap())
            nc.sync.dma_start(out=b2sb[:, :], in_=b2.ap())
            nc.sync.dma_start(out=b2hi[64:64 + NCLASS, :], in_=b2.ap())
            nc.sync.dma_start(out=w2st[:, :], in_=w2.ap())
            nc.vector.tensor_copy(w1bf[:, :, :], w1st[:, :, :])
            nc.vector.tensor_copy(w2bf[:, :], w2st[:, :])
            make_identity(nc, ident[:, :])
            make_identity(nc, ident2[64:64 + NCLASS, :])

            # ---- S phase: S_i = x_i @ W1 (stationary x^T chunks) ----
            with tc.tile_pool(name="spsum", bufs=2, space="PSUM") as spsum:
                for nci in range(MC):
                    ps = spsum.tile([128, NHID], F32, tag="ps")
                    for d in range(DF):
                        nc.tensor.matmul(
                            ps[:, :],
                            xtb[:, d, nci * 128:(nci + 1) * 128],
                            w1bf[:, d, :],
                            start=(d == 0), stop=(d == DF - 1),
                        )
                    nc.vector.tensor_copy(sloc[:, nci, :], ps[:, :])
            nc.sync.dma_start(
                out=s_in.rearrange("(a p) f -> p a f", p=128),
                in_=sloc[:, :, :],
            )
            nc.gpsimd.collective_compute(
                "AllGather", mybir.AluOpType.bypass, replica_groups=rg,
                ins=[s_in[:, :]], outs=[s_out[:, :]],
            )
            for kk in range(ADJ_EARLY, K // KK):
                adj_dma(kk)
            nc.sync.dma_start(
                out=sres[:, :, :],
                in_=s_out.rearrange("(k p) f -> p k f", p=128),
            )

            # ---- layer 1: hT += S_k^T @ adjT_k (DMA-gated on adj chunks) ----
            with tc.tile_pool(name="hpsum", bufs=1, space="PSUM") as hp:
                ph = [hp.tile([128, MW], F32, name=f"ph{m}") for m in range(MH)]
                for k in range(K):
                    for mh in range(MH):
                        nc.tensor.matmul(
                            ph[mh][:, :],
                            sres[:, k, :],
                            adjres[:, k * M + mh * MW:k * M + (mh + 1) * MW],
                            start=(k == 0), stop=(k == K - 1),
                        )
                for mh in range(MH):
                    nc.scalar.activation(
                        hT[:, mh * MW:(mh + 1) * MW], ph[mh][:, :],
                        mybir.ActivationFunctionType.Relu,
                        bias=b1sb[:, 0:1], scale=1.0,
                    )

            # ---- z_i = h_i @ W2 ----
            with tc.tile_pool(name="zpsum", bufs=2, space="PSUM") as zp:
                for mc in range(MC):
                    pz = zp.tile([128, NCLASS], F32, tag="pz")
                    nc.tensor.matmul(
                        pz[:, :],
                        hT[:, mc * 128:(mc + 1) * 128],
                        w2bf[:, :],
                        start=True, stop=True,
                    )
                    nc.vector.tensor_copy(zloc[:, mc, :], pz[:, :])
            nc.sync.dma_start(
                out=z_in.rearrange("(a p) c -> p a c", p=128),
                in_=zloc[:, :, :],
            )

            # ---- PE warm-keeper: discardable matmuls spanning the z-AllGather
            # gap so HAM keeps the PE at full clock for layer 2. ----
            with tc.tile_pool(name="wpsum", bufs=1, space="PSUM") as wp:
                wps = wp.tile([128, 128], F32)
                for i in range(WARM_N):
                    nc.tensor.matmul(
                        wps[:, :],
                        w1bf[:, 0, :],
                        hT[:, 0:128],
                        start=True, stop=True,
                    )

            nc.gpsimd.collective_compute(
                "AllGather", mybir.AluOpType.bypass, replica_groups=rg,
                ins=[z_in[:, :]], outs=[z_out[:, :]],
            )
            nc.sync.dma_start(
                out=zres[:, :, :],
                in_=z_out.rearrange("(k p) c -> p k c", p=128),
            )

            # ---- layer 2: logitsT += z_k^T @ adjT_k ----
            assert MH == 2
            with tc.tile_pool(name="lpsum", bufs=1, space="PSUM") as lp:
                if L2_COLTILE:
                    # one PSUM bank, m-block 0 -> partitions 0..39 (col-group
                    # 0/1), m-block 1 -> partitions 64..103 (col-group 2/3);
                    # the two matmuls per k stream concurrently.
                    pl = lp.tile([128, MW], F32)
                    for k in range(K):
                        nc.tensor.matmul(
                            pl[0:NCLASS, :],
                            zres[:, k, :],
                            adjres[:, k * M:k * M + MW],
                            start=(k == 0), stop=(k == K - 1),
                            tile_position=(0, 0), skip_group_check=True,
                        )
                        nc.tensor.matmul(
                            pl[64:64 + NCLASS, :],
                            zres[:, k, :],
                            adjres[:, k * M + MW:k * M + 2 * MW],
                            start=(k == 0), stop=(k == K - 1),
                            tile_position=(0, 64), skip_group_check=True,
                        )
                    nc.scalar.activation(
                        lTsb[:, 0:MW], pl[0:NCLASS, :],
                        mybir.ActivationFunctionType.Identity,
                        bias=b2sb[:, 0:1], scale=1.0,
                    )
                    nc.scalar.activation(
                        lThi[64:64 + NCLASS, :], pl[64:64 + NCLASS, :],
                        mybir.ActivationFunctionType.Identity,
                        bias=b2hi[64:64 + NCLASS, 0:1], scale=1.0,
                    )
                else:
                    pl = [lp.tile([NCLASS, MW], F32, name=f"pl{m}") for m in range(MH)]
                    for k in range(K):
                        for mh in range(MH):
                            nc.tensor.matmul(
                                pl[mh][:, :],
                                zres[:, k, :],
                                adjres[:, k * M + mh * MW:k * M + (mh + 1) * MW],
                                start=(k == 0), stop=(k == K - 1),
                            )
                    for mh in range(MH):
                        nc.scalar.activation(
                            lTsb[:, mh * MW:(mh + 1) * MW] if mh == 0
                            else lThi[64:64 + NCLASS, :],
                            pl[mh][:, :],
                            mybir.ActivationFunctionType.Identity,
                            bias=(b2sb[:, 0:1] if mh == 0
                                  else b2hi[64:64 + NCLASS, 0:1]),
                            scale=1.0,
                        )

            # ---- log_softmax over classes, batched per activation function so
            # the scalar engine loads each ACT table once ----
            with (
                tc.tile_pool(name="smp", bufs=1, space="PSUM") as smp,
                tc.tile_pool(name="sms", bufs=1) as sms,
            ):
                ptrs = [smp.tile([128, NCLASS], F32, name=f"ptr{m}") for m in range(MC)]
                mx = sms.tile([128, MC], F32)
                ssum = sms.tile([128, MC], F32)
                lse = sms.tile([128, MC], F32)
                bias2 = sms.tile([128, MC], F32)
                esc = sms.tile([128, NCLASS], F32)
                half = MC // 2
                for mc in range(MC):
                    if mc < half:
                        nc.tensor.transpose(
                            ptrs[mc][:, :], lTsb[:, mc * 128:(mc + 1) * 128],
                            ident[0:NCLASS, 0:NCLASS],
                        )
                    else:
                        nc.tensor.transpose(
                            ptrs[mc][:, :],
                            lThi[64:64 + NCLASS, (mc - half) * 128:(mc - half + 1) * 128],
                            ident2[64:64 + NCLASS, :],
                        )
                for mc in range(MC):
                    nc.vector.tensor_reduce(
                        mx[:, mc:mc + 1], ptrs[mc][:, :], axis=mybir.AxisListType.X,
                        op=mybir.AluOpType.max, negate=True,
                    )
                for mc in range(MC):
                    nc.scalar.activation(
                        esc[:, :], ptrs[mc][:, :], mybir.ActivationFunctionType.Exp,
                        bias=mx[:, mc:mc + 1], scale=1.0,
                        accum_out=ssum[:, mc:mc + 1],
                    )
                nc.scalar.activation(
                    lse[:, :], ssum[:, :], mybir.ActivationFunctionType.Ln,
                )
                nc.vector.tensor_sub(bias2[:, :], mx[:, :], lse[:, :])
                for mc in range(MC):
                    nc.vector.tensor_scalar_add(
                        osb[:, mc, :], ptrs[mc][:, :], bias2[:, mc:mc + 1],
                    )
            nc.sync.dma_start(
                out=out_ext.ap().rearrange("(a p) c -> p a c", p=128),
                in_=osb[:, :, :],
            )

    nc.compile()
    return nc


_NC_CACHE = {}


def _get_nc(n_total: int):
    if n_total not in _NC_CACHE:
        _NC_CACHE[n_total] = build(n_total)
    return _NC_CACHE[n_total]


def _permute_rows(a: np.ndarray, kk: int) -> np.ndarray:
    """Reorder rows so the device's "(p a) m" DMA layout reconstructs the
    natural "(a p) m" k-chunk layout with KK*4KB-contiguous descriptors."""
    n, m = a.shape
    nblk = n // (128 * kk)
    return np.ascontiguousarray(
        a.reshape(nblk, kk, 128, m).transpose(0, 2, 1, 3).reshape(n, m)
    )


def make_in_maps(x, adj, W1, b1, W2, b2):
    n_total = x.shape[0]
    m = n_total // NCORES
    in_maps = []
    for i in range(NCORES):
        r0 = i * m
        in_maps.append({
            "xt": np.ascontiguousarray(x[r0:r0 + m].T),
            "adjc": _permute_rows(np.ascontiguousarray(adj[:, r0:r0 + m]), KK),
            "w1": np.ascontiguousarray(W1),
            "b1": np.ascontiguousarray(b1.reshape(NHID, 1)),
            "w2": np.ascontiguousarray(W2),
            "b2": np.ascontiguousarray(b2.reshape(NCLASS, 1)),
        })
    return in_maps


def kernel(x, adj, W1, b1, W2, b2):
    x = np.asarray(x, dtype=np.float32)
    adj = np.asarray(adj, dtype=np.float32)
    W1 = np.asarray(W1, dtype=np.float32)
    b1 = np.asarray(b1, dtype=np.float32)
    W2 = np.asarray(W2, dtype=np.float32)
    b2 = np.asarray(b2, dtype=np.float32)
    nc = _get_nc(x.shape[0])
    in_maps = make_in_maps(x, adj, W1, b1, W2, b2)
    res = run_bass_kernel_spmd(nc, in_maps, list(range(NCORES)))
    return np.concatenate([res.results[i]["out"] for i in range(NCORES)], axis=0)
